# revision 35
# baseline (speedup 1.0000x reference)
"""Trainium2 Bass kernel for additive-relu attention (raw bass, explicit sync).

Reference computation (B=2, N=512, C=256):
    q, k, v = x @ Wq.T, x @ Wk.T, x @ Wv.T          # [B, N, C]
    score[b,i,j] = sum_d relu(q[b,i,d] + k[b,j,d])  # [B, N, N]
    attn = softmax(score, axis=-1)
    out = (attn @ v) @ Wp.T + bp

Sharding: data-parallel over (batch, query-block-of-128) -> 8 cores.  Each
core receives its batch's x ROTATED so its 128 queries are rows 0:128
(softmax and attn@v are invariant to a consistent key permutation), runs a
flash-style kernel over all 512 keys, and writes its [128, 256] output block.

Per-core dataflow:
  PRE : DMA x/W (k and q first); PE-transpose -> xT and WTk/WTq; project
        kT [d, keys] into dedicated PSUM banks (ACT reads it there in fp32;
        DVE reads an fp16 SBUF copy in its 4x mode) and qT (fp32).
  MAIN: per (query q, d-half h): R = relu(kT_h + qT_h[:, q]) in fp16 on DVE
        (tensor_scalar add+max, 4x mode; fp32 scalar) and ACT (Relu with
        fp32 bias from PSUM); d-reduction on the PE via col-tiled matmuls
        (batched dispatch, 4 col-groups) with shifted one-hot-column ones
        windows, accumulating S [128 queries, 512 keys] in PSUM fp32.
  TAIL: WTv/WTp + V projection (deferred); softmax (reduce_max(negate) ->
        exp(bias=-max, accum_out)); 1/r folded into a diagonal used as the
        rhs of the U-transpose (fp16); attn @ V (fp16) and the output
        projection + bias; final transpose; DMA out.

Raw bass with explicit semaphores (Tile's auto-sync emits multi-wait
instructions this walrus rejects); every wait is a standalone instruction.
Producers use separate R rings so neither elementwise engine can stall the
other through ring-slot reuse.
"""

import numpy as np

import concourse.bass as bass
import concourse.mybir as mybir
from concourse.bass_utils import run_bass_kernel_spmd

B, N, C = 2, 512, 256
P = 128
NCORES = 8
NR_V = 10                      # DVE R ring slots
NR_A = 5                       # ACT R ring slots
F32 = mybir.dt.float32
F32R = mybir.dt.float32r
F16 = mybir.dt.float16

AXT = mybir.ActivationFunctionType
ALU = mybir.AluOpType

NQH = 2 * P                    # (query, half) elementwise ops per core


def _use_dve(idx: int) -> bool:
    # DVE fp16 op ~263ns vs ACT ~609ns -> ~70% of ops on DVE
    return idx % 17 not in (2, 5, 8, 11, 14)


# rank[i] = 1-based count of same-engine ops <= i; issue list per engine
_DVE_RANK, _ACT_RANK = [], []
DVE_ISSUES, ACT_ISSUES = [], []
for _i in range(NQH):
    if _use_dve(_i):
        DVE_ISSUES.append(_i)
    else:
        ACT_ISSUES.append(_i)
    _DVE_RANK.append(len(DVE_ISSUES))
    _ACT_RANK.append(len(ACT_ISSUES))
N_DVE_R, N_ACT_R = len(DVE_ISSUES), len(ACT_ISSUES)


def _rinc_count(rank, n_total):
    """Producer sem count visible after `rank` ops with inc-per-4 (+final)."""
    return rank // 4 + (1 if rank == n_total and rank % 4 != 0 else 0)


class EngState:
    """Tracks per-engine observed sem thresholds to elide covered waits."""

    def __init__(self, eng):
        self.eng = eng
        self.seen = {}

    def wait(self, sem, thr):
        if self.seen.get(sem.name, -1) >= thr:
            return
        self.eng.wait_ge(sem, thr)
        self.seen[sem.name] = thr


def _build_body(nc, xb, wq, wk, wv, wp, bp, ident_d, onesw_d, out_d):
    ident_h = nc.alloc_sbuf_tensor("ident_sb", [P, P], F32)
    ones_h = nc.alloc_sbuf_tensor("ones_shift", [P, 64], F16)
    xt_h = nc.alloc_sbuf_tensor("xt", [P, 4, C], F32)
    w_h = {n: nc.alloc_sbuf_tensor(f"w_{n}", [P, 2, C], F32) for n in "qkvp"}
    bpt_h = nc.alloc_sbuf_tensor("bpt", [P, 2], F32)
    xT_h = nc.alloc_sbuf_tensor("xT", [P, 2, N], F32R)
    WT_h = {n: nc.alloc_sbuf_tensor(f"WT_{n}", [P, 2, C], F32R) for n in "qkvp"}
    kT_h = nc.alloc_sbuf_tensor("kT", [P, 2, N], F16)
    qT_h = nc.alloc_sbuf_tensor("qT", [P, 2, P], F32)
    V_h = nc.alloc_sbuf_tensor("V", [P, 4, C], F16)
    Rv_h = nc.alloc_sbuf_tensor("Rv", [P, NR_V, N], F16)
    Ra_h = nc.alloc_sbuf_tensor("Ra", [P, NR_A, N], F16)
    U_h = nc.alloc_sbuf_tensor("U", [P, N], F16)
    Dm_h = nc.alloc_sbuf_tensor("Dm", [P, P], F16)
    attnT_h = nc.alloc_sbuf_tensor("attnT", [P, N], F16)
    OT_h = nc.alloc_sbuf_tensor("OT", [P, 2, P], F16)
    WT16p_h = nc.alloc_sbuf_tensor("WT16p", [P, 2, C], F16)
    o2b_h = nc.alloc_sbuf_tensor("o2b", [P, 2, P], F32)
    fin_h = nc.alloc_sbuf_tensor("fin", [P, C], F32)
    negmx_h = nc.alloc_sbuf_tensor("negmx", [P, 1], F32)
    rsum_h = nc.alloc_sbuf_tensor("rsum", [P, 1], F32)
    rrec_h = nc.alloc_sbuf_tensor("rrec", [P, 1], F32)
    scr_h = nc.alloc_sbuf_tensor("scr", [P, 1], F32)

    psA_h = nc.alloc_psum_tensor("psA", [P, N], F32)
    psB_h = nc.alloc_psum_tensor("psB", [P, N], F32)
    psS_h = nc.alloc_psum_tensor("psS", [P, N], F32)
    psK_h = nc.alloc_psum_tensor("psK", [P, 2, N], F32)
    psV_h = nc.alloc_psum_tensor("psV", [P, 2, N], F32)

    ident, ones, xt, bpt = ident_h.ap(), ones_h.ap(), xt_h.ap(), bpt_h.ap()
    wts = {n: h.ap() for n, h in w_h.items()}
    xT, kT, qT, V = xT_h.ap(), kT_h.ap(), qT_h.ap(), V_h.ap()
    Rv, Ra = Rv_h.ap(), Ra_h.ap()
    WT = {n: h.ap() for n, h in WT_h.items()}
    U, Dm, attnT, OT = U_h.ap(), Dm_h.ap(), attnT_h.ap(), OT_h.ap()
    WT16p = WT16p_h.ap()
    o2b, fin, scr = o2b_h.ap(), fin_h.ap(), scr_h.ap()
    negmx, rsum, rrec = negmx_h.ap(), rsum_h.ap(), rrec_h.ap()
    psA, psB, psS, psK = psA_h.ap(), psB_h.ap(), psS_h.ap(), psK_h.ap()
    psV = psV_h.ap()

    # ---- semaphore plan ----
    # SP queue: x halves (sDh 32), ident (sDi 16), ones (sDon 16)
    # ACT queue: wk (sDk 16), wq (sDq 16), wv+wp (sDvp 32), bias (sDb 16)
    # PE groups (sPE): xT 1-2, WTk 3-4, kT 5-6 (psK), WTq 7-8, qT 9-10 |
    # MAIN: inc per batch-of-4 -> 10+64=74 | WTv 75-76, WTp 77-78, V 79-82
    # (disjoint psum regions), ATT 83, OT 84-85, O2 86-87, FIN 88
    PE_XT = [1, 2]
    PE_WTK = [3, 4]
    PE_KT = [5, 6]
    PE_WTQ = [7, 8]
    PE_QT = [9, 10]
    PE_WTV = [11, 12]
    PE_WTP = [13, 14]
    PE_V = [15, 16, 17, 18]
    PE_PREC = 18
    PE_MAIN_DONE = PE_PREC + NQH // 4
    PE_ATT = PE_MAIN_DONE + 1
    PE_OT = [PE_ATT + 1, PE_ATT + 2]
    PE_O2 = [PE_ATT + 3, PE_ATT + 4]
    PE_FIN = PE_ATT + 5

    def pe_main_thr(i):
        """sPE count once the batch containing main MM issue i completes."""
        return PE_PREC + i // 4 + 1

    # DVE stream (sV): xT1c, WTk1c, kT1c, WTq1c, qT1c (1..5), R ops
    # (inc per 2), negmx, WTv1c, WTp1c, V copies x4, rrec, Dm, OTc x2,
    # o2b x2
    V_WTV = 6
    V_WTP = 7
    V_PREC = 7
    V_RINC = _rinc_count(N_DVE_R, N_DVE_R)
    V_VC_D = [V_PREC + V_RINC + 1, V_PREC + V_RINC + 2]
    V_NEGMX = V_VC_D[1] + 1
    V_RREC = V_NEGMX + 1
    V_DM = V_RREC + 1
    V_OTC = [V_DM + 1, V_DM + 2]
    V_O2B = [V_DM + 3, V_DM + 4]

    # ACT stream (sA): dummy exp (1), xT0c, WTk0c, kT0c, WTq0c, qT0c
    # (2..6), R ops (inc per 2), WTv0c, WTp0c, exp, attnTc, finc
    A_WTV = 7
    A_WTP = 8
    A_PREC = 8
    A_RINC = _rinc_count(N_ACT_R, N_ACT_R)
    A_VC = [A_PREC + A_RINC + 1, A_PREC + A_RINC + 2]
    A_EXP = A_VC[1] + 1
    A_ATTC = A_EXP + 1
    A_FINC = A_EXP + 2

    with (
        nc.semaphore("sDi") as sDi,
        nc.semaphore("sDh") as sDh,
        nc.semaphore("sDk") as sDk,
        nc.semaphore("sDq") as sDq,
        nc.semaphore("sDon") as sDon,
        nc.semaphore("sDvp") as sDvp,
        nc.semaphore("sDb") as sDb,
        nc.semaphore("sDo") as sDo,
        nc.semaphore("sPE") as sPE,
        nc.semaphore("sV") as sV,
        nc.semaphore("sA") as sA,
        nc.Block(no_gpsimd_drain=True) as block,
    ):

        @block.sync
        def _(sync):
            sync.dma_start(out=xt, in_=xb.rearrange("(t p) c -> p t c", p=P)
                           ).then_inc(sDh, 16)
            sync.dma_start(out=ident, in_=ident_d).then_inc(sDi, 16)
            sync.dma_start(out=wts["k"],
                           in_=wk.rearrange("(t p) c -> p t c", p=P)
                           ).then_inc(sDk, 16)
            sync.dma_start(out=ones, in_=onesw_d).then_inc(sDon, 16)
            sync.wait_ge(sA, A_FINC)
            sync.dma_start(out=out_d, in_=fin).then_inc(sDo, 16)
            sync.wait_ge(sDo, 16)

        @block.tensor
        def _(tensor):
            E = EngState(tensor)
            E.wait(sDi, 16)
            E.wait(sDh, 16)
            for h in range(2):          # xT transposes
                ps = psA if h == 0 else psB
                for t in range(4):
                    mm = nc.tensor.transpose(
                        ps[:, t * P : (t + 1) * P],
                        xt[:, t, h * P : (h + 1) * P], ident)
                mm.then_inc(sPE, 1)
            E.wait(sDk, 16)
            for h in range(2):          # WTk transposes
                ps = psA if h == 0 else psB
                E.wait(sA if h == 0 else sV, 2 if h == 0 else 1)
                for t in range(2):
                    mm = nc.tensor.transpose(
                        ps[:, t * P : (t + 1) * P],
                        wts["k"][:, t, h * P : (h + 1) * P], ident)
                mm.then_inc(sPE, 1)
            # kT projection into dedicated PSUM banks (no WAR)
            E.wait(sA, 3)
            E.wait(sV, 2)
            for h in range(2):
                for kc in range(2):
                    mm = nc.tensor.matmul(
                        psK[:, h, :], lhsT=WT["k"][:, kc, h * P : (h + 1) * P],
                        rhs=xT[:, kc, :], start=(kc == 0), stop=(kc == 1))
                mm.then_inc(sPE, 1)
            E.wait(sDq, 16)
            for h in range(2):          # WTq transposes
                ps = psA if h == 0 else psB
                for t in range(2):
                    mm = nc.tensor.transpose(
                        ps[:, t * P : (t + 1) * P],
                        wts["q"][:, t, h * P : (h + 1) * P], ident)
                mm.then_inc(sPE, 1)
            E.wait(sA, 5)               # WTq0c (+psA WAR)
            E.wait(sV, 4)               # WTq1c (+psB WAR)
            for h in range(2):          # qT projection
                ps = psA[:, 0:P] if h == 0 else psB[:, 0:P]
                for kc in range(2):
                    mm = nc.tensor.matmul(
                        ps, lhsT=WT["q"][:, kc, h * P : (h + 1) * P],
                        rhs=xT[:, kc, 0:P], start=(kc == 0), stop=(kc == 1))
                mm.then_inc(sPE, 1)
            # WTv / WTp transposes (psA/psB free: last readers were the
            # qT copies, long done)
            E.wait(sDvp, 32)
            for name, thr in (("v", (6, 5)), ("p", (A_WTV, V_WTV))):
                for h in range(2):
                    ps = psA if h == 0 else psB
                    E.wait(sA if h == 0 else sV, thr[h])
                    for t in range(2):
                        mm = nc.tensor.transpose(
                            ps[:, t * P : (t + 1) * P],
                            wts[name][:, t, h * P : (h + 1) * P], ident)
                    mm.then_inc(sPE, 1)
            # V projection: four disjoint psum regions, no copy-WAR
            for jc in range(4):
                ps = ((psA if jc == 0 else psB)[:, 0:C] if jc < 2
                      else psV[:, jc - 2, 0:C])
                if jc == 0:
                    E.wait(sA, A_WTP)
                    E.wait(sV, V_WTP)
                for kc in range(2):
                    mm = nc.tensor.matmul(
                        ps, lhsT=xT[:, kc, jc * P : (jc + 1) * P],
                        rhs=WT["v"][:, kc, :], start=(kc == 0), stop=(kc == 1))
                mm.then_inc(sPE, 1)
            # main: 256 one-hot reduction matmuls, col-tiled, dispatched in
            # batches of 4 (one per col-group) so the array streams overlap
            E.wait(sDon, 16)
            for t in range(NQH // 4):
                batch = range(4 * t, 4 * t + 4)
                dr = [_DVE_RANK[i] for i in batch if _use_dve(i)]
                ar = [_ACT_RANK[i] for i in batch if not _use_dve(i)]
                if dr:
                    E.wait(sV, V_PREC + (max(dr) + 3) // 4)
                if ar:
                    E.wait(sA, A_PREC + (max(ar) + 3) // 4)
                for i in batch:
                    sh, g = divmod(i, 4)
                    s, h = divmod(sh, 2)
                    if _use_dve(i):
                        r = Rv[:, (_DVE_RANK[i] - 1) % NR_V, :]
                    else:
                        r = Ra[:, (_ACT_RANK[i] - 1) % NR_A, :]
                    mm = nc.tensor.matmul(
                        psS[32 * g : 32 * (g + 1), :],
                        lhsT=ones[:, 32 - s : 64 - s],
                        rhs=r,
                        start=(s == 0 and h == 0),
                        stop=(s == 31 and h == 1),
                        tile_position=(0, 32 * g),
                        skip_group_check=True,
                    )
                mm.then_inc(sPE, 1)
            # attnT = U^T @ diag(1/r) (fp16), into psB (V1/V3 copied out)
            E.wait(sV, V_DM)
            E.wait(sA, A_EXP)
            for t in range(4):
                mm = nc.tensor.matmul(
                    psB[:, t * P : (t + 1) * P],
                    lhsT=U[:, t * P : (t + 1) * P], rhs=Dm,
                    start=True, stop=True)
            mm.then_inc(sPE, 1)
            # OT[m] = (attn @ V).T halves (fp16), into psA
            E.wait(sA, A_ATTC)
            E.wait(sV, V_VC_D[1])
            for m in range(2):
                if m == 1:
                    E.wait(sV, V_OTC[0])
                for jc in range(4):
                    mm = nc.tensor.matmul(
                        psA[:, m * P : (m + 1) * P],
                        lhsT=V[:, jc, m * P : (m + 1) * P],
                        rhs=attnT[:, jc * P : (jc + 1) * P],
                        start=(jc == 0), stop=(jc == 3))
                mm.then_inc(sPE, 1)
            # out2T halves = WpT16 @ OT (fp16), into psB
            E.wait(sV, V_OTC[1])
            E.wait(sA, A_ATTC)
            for m in range(2):
                if m == 1:
                    E.wait(sV, V_O2B[0])
                for kc in range(2):
                    mm = nc.tensor.matmul(
                        psB[:, m * P : (m + 1) * P],
                        lhsT=WT16p[:, kc, m * P : (m + 1) * P],
                        rhs=OT[:, kc, :], start=(kc == 0), stop=(kc == 1))
                mm.then_inc(sPE, 1)
            # final transpose [dp, i] -> [i, dp] into psA[:, 256:512]
            E.wait(sV, V_O2B[1])
            for m in range(2):
                mm = nc.tensor.transpose(
                    psA[:, C + m * P : C + (m + 1) * P], o2b[:, m, :], ident)
            mm.then_inc(sPE, 1)

        @block.vector
        def _(vector):
            E = EngState(vector)
            for thr, dst, srcap in (
                (PE_XT[1], xT[:, 1, :], psB),
                (PE_WTK[1], WT["k"][:, 1, :], psB[:, 0:C]),
                (PE_KT[1], kT[:, 1, :], psK[:, 1, :]),
                (PE_WTQ[1], WT["q"][:, 1, :], psB[:, 0:C]),
                (PE_QT[1], qT[:, 1, :], psB[:, 0:P]),
                (PE_WTV[1], WT["v"][:, 1, :], psB[:, 0:C]),
                (PE_WTP[1], WT16p[:, 1, :], psB[:, 0:C]),
            ):
                E.wait(sPE, thr)
                nc.vector.tensor_copy(dst, srcap).then_inc(sV, 1)
            # R ops (scalar operands prefetched -> cross + self sync)
            E.wait(sA, 6)
            E.wait(sV, 5)
            for i in range(NQH):        # R (DVE share)
                if not _use_dve(i):
                    continue
                sh, g = divmod(i, 4)
                s, h = divmod(sh, 2)
                q = 32 * g + s
                rank = _DVE_RANK[i]
                if rank > NR_V:
                    E.wait(sPE, pe_main_thr(DVE_ISSUES[rank - 1 - NR_V]))
                ins = nc.vector.tensor_scalar(
                    out=Rv[:, (rank - 1) % NR_V, :], in0=kT[:, h, :],
                    scalar1=qT[:, h, q : q + 1], scalar2=0.0,
                    op0=ALU.add, op1=ALU.max,
                )
                if rank % 4 == 0 or rank == N_DVE_R:
                    ins.then_inc(sV, 1)
            for jc in range(2):         # V0/V1 copies (fp16)
                E.wait(sPE, PE_V[jc])
                nc.vector.tensor_copy(
                    V[:, jc, :], (psA if jc == 0 else psB)[:, 0:C]
                ).then_inc(sV, 1)
            E.wait(sPE, PE_MAIN_DONE)
            nc.vector.tensor_reduce(
                out=negmx, in_=psS, axis=mybir.AxisListType.X,
                op=ALU.max, negate=True,
            ).then_inc(sV, 1)
            E.wait(sA, A_EXP)
            nc.vector.reciprocal(rrec, rsum).then_inc(sV, 1)
            E.wait(sV, V_RREC)          # rrec is a prefetched scalar below
            nc.vector.tensor_scalar(
                out=Dm, in0=ident, scalar1=rrec, scalar2=None, op0=ALU.mult,
            ).then_inc(sV, 1)
            for m in range(2):          # OT copies (fp16)
                E.wait(sPE, PE_OT[m])
                nc.vector.tensor_copy(OT[:, m, :],
                                      psA[:, m * P : (m + 1) * P]
                                      ).then_inc(sV, 1)
            E.wait(sDb, 16)
            for m in range(2):          # out2T + bias -> sbuf
                E.wait(sPE, PE_O2[m])
                nc.vector.tensor_scalar(
                    out=o2b[:, m, :], in0=psB[:, m * P : (m + 1) * P],
                    scalar1=bpt[:, m : m + 1], scalar2=None, op0=ALU.add,
                ).then_inc(sV, 1)

        @block.scalar
        def _(scalar):
            E = EngState(scalar)
            # secondary HWDGE queue: wq, wv, wp, bias
            nc.scalar.dma_start(out=wts["q"],
                                in_=wq.rearrange("(t p) c -> p t c", p=P)
                                ).then_inc(sDq, 16)
            for name, w in (("v", wv), ("p", wp)):
                nc.scalar.dma_start(out=wts[name],
                                    in_=w.rearrange("(t p) c -> p t c", p=P)
                                    ).then_inc(sDvp, 16)
            with nc.allow_non_contiguous_dma(reason="1KB bias load"):
                nc.scalar.dma_start(out=bpt,
                                    in_=bp.rearrange("(h p) -> p h", p=P)
                                    ).then_inc(sDb, 16)
            # preload the exp table set (relu rides along)
            E.wait(sDi, 16)
            nc.scalar.activation(out=scr, in_=ident[:, 0:1], func=AXT.Exp
                                 ).then_inc(sA, 1)
            for thr, dst, srcap in (
                (PE_XT[0], xT[:, 0, :], psA),
                (PE_WTK[0], WT["k"][:, 0, :], psA[:, 0:C]),
                (PE_KT[0], kT[:, 0, :], psK[:, 0, :]),
                (PE_WTQ[0], WT["q"][:, 0, :], psA[:, 0:C]),
                (PE_QT[0], qT[:, 0, :], psA[:, 0:P]),
                (PE_WTV[0], WT["v"][:, 0, :], psA[:, 0:C]),
                (PE_WTP[0], WT16p[:, 0, :], psA[:, 0:C]),
            ):
                E.wait(sPE, thr)
                nc.scalar.copy(dst, srcap).then_inc(sA, 1)
            # R ops: in_ = fp32 kT straight from PSUM (faster + exact add)
            E.wait(sV, 5)
            E.wait(sA, A_PREC)
            E.wait(sPE, PE_KT[1])
            for i in range(NQH):        # R (ACT share)
                if _use_dve(i):
                    continue
                sh, g = divmod(i, 4)
                s, h = divmod(sh, 2)
                q = 32 * g + s
                rank = _ACT_RANK[i]
                if rank > NR_A:
                    E.wait(sPE, pe_main_thr(ACT_ISSUES[rank - 1 - NR_A]))
                ins = nc.scalar.activation(
                    out=Ra[:, (rank - 1) % NR_A, :], in_=psK[:, h, :],
                    func=AXT.Relu, bias=qT[:, h, q : q + 1], scale=1.0,
                )
                if rank % 4 == 0 or rank == N_ACT_R:
                    ins.then_inc(sA, 1)
            for jc in range(2, 4):      # V2/V3 copies (fp16)
                E.wait(sPE, PE_V[jc])
                nc.scalar.copy(V[:, jc, :], psV[:, jc - 2, 0:C]).then_inc(sA, 1)
            E.wait(sPE, PE_MAIN_DONE)
            E.wait(sV, V_NEGMX)
            nc.scalar.activation(
                out=U, in_=psS, func=AXT.Exp, bias=negmx, scale=1.0,
                accum_out=rsum,
            ).then_inc(sA, 1)
            E.wait(sPE, PE_ATT)
            nc.scalar.copy(attnT, psB).then_inc(sA, 1)
            E.wait(sPE, PE_FIN)
            nc.scalar.copy(fin, psA[:, C : 2 * C]).then_inc(sA, 1)



_PROGRAM = None


def build_program():
    global _PROGRAM
    if _PROGRAM is not None:
        return _PROGRAM
    nc = bass.Bass(
        "TRN2", target_bir_lowering=False, debug=False, num_devices=NCORES
    )
    xb = nc.dram_tensor("xb", [N, C], F32, kind="ExternalInput")
    wq = nc.dram_tensor("wq", [C, C], F32, kind="ExternalInput")
    wk = nc.dram_tensor("wk", [C, C], F32, kind="ExternalInput")
    wv = nc.dram_tensor("wv", [C, C], F32, kind="ExternalInput")
    wp = nc.dram_tensor("wp", [C, C], F32, kind="ExternalInput")
    bp = nc.dram_tensor("bp", [C], F32, kind="ExternalInput")
    ident = nc.dram_tensor("ident", [P, P], F32, kind="ExternalInput")
    onesw = nc.dram_tensor("onesw", [P, 64], F16, kind="ExternalInput")
    out = nc.dram_tensor("out", [P, C], F32, kind="ExternalOutput")
    _build_body(nc, xb.ap(), wq.ap(), wk.ap(), wv.ap(), wp.ap(), bp.ap(),
                ident.ap(), onesw.ap(), out.ap())
    _PROGRAM = nc
    return nc


def make_in_maps(x, Wq, Wk, Wv, Wp, bp):
    """Per-core inputs: core = (batch, query-block); x rotated so the core's
    query block is rows 0:128."""
    x = np.ascontiguousarray(np.asarray(x, dtype=np.float32))
    onesw = np.zeros((P, 64), dtype=np.float16)
    onesw[:, 32] = 1.0
    common = {
        "ident": np.eye(P, dtype=np.float32),
        "onesw": onesw,
        "wq": np.ascontiguousarray(np.asarray(Wq, dtype=np.float32)),
        "wk": np.ascontiguousarray(np.asarray(Wk, dtype=np.float32)),
        "wv": np.ascontiguousarray(np.asarray(Wv, dtype=np.float32)),
        "wp": np.ascontiguousarray(np.asarray(Wp, dtype=np.float32)),
        "bp": np.ascontiguousarray(np.asarray(bp, dtype=np.float32)),
    }
    in_maps = []
    for core in range(NCORES):
        b, qb = divmod(core, NCORES // B)
        xrot = np.ascontiguousarray(np.roll(x[b], -qb * P, axis=0))
        in_maps.append({"xb": xrot, **common})
    return in_maps


def assemble(results):
    out = np.zeros((B, N, C), dtype=np.float32)
    for core in range(NCORES):
        b, qb = divmod(core, NCORES // B)
        out[b, qb * P : (qb + 1) * P] = results[core]["out"]
    return out


def kernel(x, Wq, Wk, Wv, Wp, bp):
    nc = build_program()
    in_maps = make_in_maps(x, Wq, Wk, Wv, Wp, bp)
    res = run_bass_kernel_spmd(nc, in_maps, list(range(NCORES)))
    return assemble(res.results)


if __name__ == "__main__":
    rng = np.random.default_rng(0)
    inputs = {
        "x": rng.standard_normal((B, N, C), dtype=np.float32),
        "Wq": rng.standard_normal((C, C), dtype=np.float32) * 0.02,
        "Wk": rng.standard_normal((C, C), dtype=np.float32) * 0.02,
        "Wv": rng.standard_normal((C, C), dtype=np.float32) * 0.02,
        "Wp": rng.standard_normal((C, C), dtype=np.float32) * 0.02,
        "bp": rng.standard_normal((C,), dtype=np.float32) * 0.02,
    }
    out = kernel(**inputs)
    print(out.shape, out.dtype)


# revision 36
# speedup vs baseline: 1.0531x; 1.0531x over previous
"""Trainium2 Bass kernel for additive-relu attention (raw bass, explicit sync).

Reference computation (B=2, N=512, C=256):
    q, k, v = x @ Wq.T, x @ Wk.T, x @ Wv.T          # [B, N, C]
    score[b,i,j] = sum_d relu(q[b,i,d] + k[b,j,d])  # [B, N, N]
    attn = softmax(score, axis=-1)
    out = (attn @ v) @ Wp.T + bp

Sharding: data-parallel over (batch, query-block-of-128) -> 8 cores.  Each
core receives its batch's x ROTATED so its 128 queries are rows 0:128
(softmax and attn@v are invariant to a consistent key permutation), runs a
flash-style kernel over all 512 keys, and writes its [128, 256] output block.

Per-core dataflow:
  PRE : DMA x/W (k and q first); PE-transpose -> xT and WTk/WTq; project
        kT [d, keys] into dedicated PSUM banks (ACT reads it there in fp32;
        DVE reads an fp16 SBUF copy in its 4x mode) and qT (fp32).
  MAIN: per (query q, d-half h): R = relu(kT_h + qT_h[:, q]) in fp16 on DVE
        (tensor_scalar add+max, 4x mode; fp32 scalar) and ACT (Relu with
        fp32 bias from PSUM); d-reduction on the PE via col-tiled matmuls
        (batched dispatch, 4 col-groups) with shifted one-hot-column ones
        windows, accumulating S [128 queries, 512 keys] in PSUM fp32.
  TAIL: WTv/WTp + V projection (deferred); softmax (reduce_max(negate) ->
        exp(bias=-max, accum_out)); 1/r folded into a diagonal used as the
        rhs of the U-transpose (fp16); attn @ V (fp16) and the output
        projection + bias; final transpose; DMA out.

Raw bass with explicit semaphores (Tile's auto-sync emits multi-wait
instructions this walrus rejects); every wait is a standalone instruction.
Producers use separate R rings so neither elementwise engine can stall the
other through ring-slot reuse.
"""

import numpy as np

import concourse.bass as bass
import concourse.mybir as mybir
from concourse.bass_utils import run_bass_kernel_spmd

B, N, C = 2, 512, 256
P = 128
NCORES = 8
NR_V = 10                      # DVE R ring slots
NR_A = 5                       # ACT R ring slots
F32 = mybir.dt.float32
F32R = mybir.dt.float32r
F16 = mybir.dt.float16

AXT = mybir.ActivationFunctionType
ALU = mybir.AluOpType

NQH = 2 * P                    # (query, half) elementwise ops per core


def _use_dve(idx: int) -> bool:
    # DVE fp16 op ~263ns vs ACT ~609ns -> ~70% of ops on DVE
    return idx % 17 not in (2, 5, 8, 11, 14)


# rank[i] = 1-based count of same-engine ops <= i; issue list per engine
_DVE_RANK, _ACT_RANK = [], []
DVE_ISSUES, ACT_ISSUES = [], []
for _i in range(NQH):
    if _use_dve(_i):
        DVE_ISSUES.append(_i)
    else:
        ACT_ISSUES.append(_i)
    _DVE_RANK.append(len(DVE_ISSUES))
    _ACT_RANK.append(len(ACT_ISSUES))
N_DVE_R, N_ACT_R = len(DVE_ISSUES), len(ACT_ISSUES)


def _rinc_count(rank, n_total):
    """Producer sem count visible after `rank` ops with inc-per-2 (+final)."""
    return rank // 2 + (1 if rank == n_total and rank % 2 == 1 else 0)


class EngState:
    """Tracks per-engine observed sem thresholds to elide covered waits."""

    def __init__(self, eng):
        self.eng = eng
        self.seen = {}

    def wait(self, sem, thr):
        if self.seen.get(sem.name, -1) >= thr:
            return
        self.eng.wait_ge(sem, thr)
        self.seen[sem.name] = thr


def _build_body(nc, xb, wq, wk, wv, wp, bp, ident_d, onesw_d, out_d):
    ident_h = nc.alloc_sbuf_tensor("ident_sb", [P, P], F32)
    ones_h = nc.alloc_sbuf_tensor("ones_shift", [P, 64], F16)
    xt_h = nc.alloc_sbuf_tensor("xt", [P, 4, C], F32)
    w_h = {n: nc.alloc_sbuf_tensor(f"w_{n}", [P, 2, C], F32) for n in "qkvp"}
    bpt_h = nc.alloc_sbuf_tensor("bpt", [P, 2], F32)
    xT_h = nc.alloc_sbuf_tensor("xT", [P, 2, N], F32R)
    WT_h = {n: nc.alloc_sbuf_tensor(f"WT_{n}", [P, 2, C], F32R) for n in "qkvp"}
    kT_h = nc.alloc_sbuf_tensor("kT", [P, 2, N], F16)
    qT_h = nc.alloc_sbuf_tensor("qT", [P, 2, P], F32)
    V_h = nc.alloc_sbuf_tensor("V", [P, 4, C], F16)
    Rv_h = nc.alloc_sbuf_tensor("Rv", [P, NR_V, N], F16)
    Ra_h = nc.alloc_sbuf_tensor("Ra", [P, NR_A, N], F16)
    U_h = nc.alloc_sbuf_tensor("U", [P, N], F16)
    Dm_h = nc.alloc_sbuf_tensor("Dm", [P, P], F16)
    attnT_h = nc.alloc_sbuf_tensor("attnT", [P, N], F16)
    OT_h = nc.alloc_sbuf_tensor("OT", [P, 2, P], F16)
    WT16p_h = nc.alloc_sbuf_tensor("WT16p", [P, 2, C], F16)
    o2b_h = nc.alloc_sbuf_tensor("o2b", [P, 2, P], F32)
    fin_h = nc.alloc_sbuf_tensor("fin", [P, C], F32)
    negmx_h = nc.alloc_sbuf_tensor("negmx", [P, 1], F32)
    rsum_h = nc.alloc_sbuf_tensor("rsum", [P, 1], F32)
    rrec_h = nc.alloc_sbuf_tensor("rrec", [P, 1], F32)
    scr_h = nc.alloc_sbuf_tensor("scr", [P, 1], F32)

    psA_h = nc.alloc_psum_tensor("psA", [P, N], F32)
    psB_h = nc.alloc_psum_tensor("psB", [P, N], F32)
    psS_h = nc.alloc_psum_tensor("psS", [P, N], F32)
    psK_h = nc.alloc_psum_tensor("psK", [P, 2, N], F32)
    psV_h = nc.alloc_psum_tensor("psV", [P, 2, N], F32)

    ident, ones, xt, bpt = ident_h.ap(), ones_h.ap(), xt_h.ap(), bpt_h.ap()
    wts = {n: h.ap() for n, h in w_h.items()}
    xT, kT, qT, V = xT_h.ap(), kT_h.ap(), qT_h.ap(), V_h.ap()
    Rv, Ra = Rv_h.ap(), Ra_h.ap()
    WT = {n: h.ap() for n, h in WT_h.items()}
    U, Dm, attnT, OT = U_h.ap(), Dm_h.ap(), attnT_h.ap(), OT_h.ap()
    WT16p = WT16p_h.ap()
    o2b, fin, scr = o2b_h.ap(), fin_h.ap(), scr_h.ap()
    negmx, rsum, rrec = negmx_h.ap(), rsum_h.ap(), rrec_h.ap()
    psA, psB, psS, psK = psA_h.ap(), psB_h.ap(), psS_h.ap(), psK_h.ap()
    psV = psV_h.ap()

    # ---- semaphore plan ----
    # SP queue: x halves (sDh 32), ident (sDi 16), ones (sDon 16)
    # ACT queue: wk (sDk 16), wq (sDq 16), wv+wp (sDvp 32), bias (sDb 16)
    # PE groups (sPE): xT 1-2, WTk 3-4, kT 5-6 (psK), WTq 7-8, qT 9-10 |
    # MAIN: inc per batch-of-4 -> 10+64=74 | WTv 75-76, WTp 77-78, V 79-82
    # (disjoint psum regions), ATT 83, OT 84-85, O2 86-87, FIN 88
    PE_XT = [1, 2]
    PE_WTK = [3, 4]
    PE_KT = [5, 6]
    PE_WTQ = [7, 8]
    PE_QT = [9, 10]
    PE_WTV = [11, 12]
    PE_WTP = [13, 14]
    PE_V = [15, 16, 17, 18]
    PE_PREC = 18
    PE_MAIN_DONE = PE_PREC + NQH // 4
    PE_ATT = PE_MAIN_DONE + 1
    PE_OT = [PE_ATT + 1, PE_ATT + 2]
    PE_O2 = [PE_ATT + 3, PE_ATT + 4]
    PE_FIN = PE_ATT + 5

    def pe_main_thr(i):
        """sPE count once the batch containing main MM issue i completes."""
        return PE_PREC + i // 4 + 1

    # DVE stream (sV): xT1c, WTk1c, kT1c, WTq1c, qT1c (1..5), R ops
    # (inc per 2), negmx, WTv1c, WTp1c, V copies x4, rrec, Dm, OTc x2,
    # o2b x2
    V_WTV = 6
    V_WTP = 7
    V_PREC = 7
    V_RINC = _rinc_count(N_DVE_R, N_DVE_R)
    V_VC_D = [V_PREC + V_RINC + 1, V_PREC + V_RINC + 2]
    V_NEGMX = V_VC_D[1] + 1
    V_RREC = V_NEGMX + 1
    V_DM = V_RREC + 1
    V_OTC = [V_DM + 1, V_DM + 2]
    V_O2B = [V_DM + 3, V_DM + 4]

    # ACT stream (sA): dummy exp (1), xT0c, WTk0c, kT0c, WTq0c, qT0c
    # (2..6), R ops (inc per 2), WTv0c, WTp0c, exp, attnTc, finc
    A_WTV = 7
    A_WTP = 8
    A_PREC = 8
    A_RINC = _rinc_count(N_ACT_R, N_ACT_R)
    A_VC = [A_PREC + A_RINC + 1, A_PREC + A_RINC + 2]
    A_EXP = A_VC[1] + 1
    A_ATTC = A_EXP + 1
    A_FINC = A_EXP + 2

    with (
        nc.semaphore("sDi") as sDi,
        nc.semaphore("sDh") as sDh,
        nc.semaphore("sDk") as sDk,
        nc.semaphore("sDq") as sDq,
        nc.semaphore("sDon") as sDon,
        nc.semaphore("sDvp") as sDvp,
        nc.semaphore("sDb") as sDb,
        nc.semaphore("sDo") as sDo,
        nc.semaphore("sPE") as sPE,
        nc.semaphore("sV") as sV,
        nc.semaphore("sA") as sA,
        nc.Block(no_gpsimd_drain=True) as block,
    ):

        @block.sync
        def _(sync):
            sync.dma_start(out=xt, in_=xb.rearrange("(t p) c -> p t c", p=P)
                           ).then_inc(sDh, 16)
            sync.dma_start(out=ident, in_=ident_d).then_inc(sDi, 16)
            sync.dma_start(out=wts["k"],
                           in_=wk.rearrange("(t p) c -> p t c", p=P)
                           ).then_inc(sDk, 16)
            sync.dma_start(out=ones, in_=onesw_d).then_inc(sDon, 16)
            sync.wait_ge(sA, A_FINC)
            sync.dma_start(out=out_d, in_=fin).then_inc(sDo, 16)
            sync.wait_ge(sDo, 16)

        @block.tensor
        def _(tensor):
            E = EngState(tensor)
            E.wait(sDi, 16)
            E.wait(sDh, 16)
            for h in range(2):          # xT transposes
                ps = psA if h == 0 else psB
                for t in range(4):
                    mm = nc.tensor.transpose(
                        ps[:, t * P : (t + 1) * P],
                        xt[:, t, h * P : (h + 1) * P], ident)
                mm.then_inc(sPE, 1)
            E.wait(sDk, 16)
            for h in range(2):          # WTk transposes
                ps = psA if h == 0 else psB
                E.wait(sA if h == 0 else sV, 2 if h == 0 else 1)
                for t in range(2):
                    mm = nc.tensor.transpose(
                        ps[:, t * P : (t + 1) * P],
                        wts["k"][:, t, h * P : (h + 1) * P], ident)
                mm.then_inc(sPE, 1)
            # kT projection into dedicated PSUM banks (no WAR)
            E.wait(sA, 3)
            E.wait(sV, 2)
            for h in range(2):
                for kc in range(2):
                    mm = nc.tensor.matmul(
                        psK[:, h, :], lhsT=WT["k"][:, kc, h * P : (h + 1) * P],
                        rhs=xT[:, kc, :], start=(kc == 0), stop=(kc == 1))
                mm.then_inc(sPE, 1)
            E.wait(sDq, 16)
            for h in range(2):          # WTq transposes
                ps = psA if h == 0 else psB
                for t in range(2):
                    mm = nc.tensor.transpose(
                        ps[:, t * P : (t + 1) * P],
                        wts["q"][:, t, h * P : (h + 1) * P], ident)
                mm.then_inc(sPE, 1)
            E.wait(sA, 5)               # WTq0c (+psA WAR)
            E.wait(sV, 4)               # WTq1c (+psB WAR)
            for h in range(2):          # qT projection
                ps = psA[:, 0:P] if h == 0 else psB[:, 0:P]
                for kc in range(2):
                    mm = nc.tensor.matmul(
                        ps, lhsT=WT["q"][:, kc, h * P : (h + 1) * P],
                        rhs=xT[:, kc, 0:P], start=(kc == 0), stop=(kc == 1))
                mm.then_inc(sPE, 1)
            # WTv / WTp transposes (psA/psB free: last readers were the
            # qT copies, long done)
            E.wait(sDvp, 32)
            for name, thr in (("v", (6, 5)), ("p", (A_WTV, V_WTV))):
                for h in range(2):
                    ps = psA if h == 0 else psB
                    E.wait(sA if h == 0 else sV, thr[h])
                    for t in range(2):
                        mm = nc.tensor.transpose(
                            ps[:, t * P : (t + 1) * P],
                            wts[name][:, t, h * P : (h + 1) * P], ident)
                    mm.then_inc(sPE, 1)
            # V projection: four disjoint psum regions, no copy-WAR
            for jc in range(4):
                ps = ((psA if jc == 0 else psB)[:, 0:C] if jc < 2
                      else psV[:, jc - 2, 0:C])
                if jc == 0:
                    E.wait(sA, A_WTP)
                    E.wait(sV, V_WTP)
                for kc in range(2):
                    mm = nc.tensor.matmul(
                        ps, lhsT=xT[:, kc, jc * P : (jc + 1) * P],
                        rhs=WT["v"][:, kc, :], start=(kc == 0), stop=(kc == 1))
                mm.then_inc(sPE, 1)
            # main: 256 one-hot reduction matmuls, col-tiled, dispatched in
            # batches of 4 (one per col-group) so the array streams overlap
            E.wait(sDon, 16)
            for t in range(NQH // 4):
                batch = range(4 * t, 4 * t + 4)
                dr = [_DVE_RANK[i] for i in batch if _use_dve(i)]
                ar = [_ACT_RANK[i] for i in batch if not _use_dve(i)]
                if dr:
                    E.wait(sV, V_PREC + (max(dr) + 1) // 2)
                if ar:
                    E.wait(sA, A_PREC + (max(ar) + 1) // 2)
                for i in batch:
                    sh, g = divmod(i, 4)
                    s, h = divmod(sh, 2)
                    if _use_dve(i):
                        r = Rv[:, (_DVE_RANK[i] - 1) % NR_V, :]
                    else:
                        r = Ra[:, (_ACT_RANK[i] - 1) % NR_A, :]
                    mm = nc.tensor.matmul(
                        psS[32 * g : 32 * (g + 1), :],
                        lhsT=ones[:, 32 - s : 64 - s],
                        rhs=r,
                        start=(s == 0 and h == 0),
                        stop=(s == 31 and h == 1),
                        tile_position=(0, 32 * g),
                        skip_group_check=True,
                    )
                mm.then_inc(sPE, 1)
            # attnT = U^T @ diag(1/r) (fp16), into psB (V1/V3 copied out)
            E.wait(sV, V_DM)
            E.wait(sA, A_EXP)
            for t in range(4):
                mm = nc.tensor.matmul(
                    psB[:, t * P : (t + 1) * P],
                    lhsT=U[:, t * P : (t + 1) * P], rhs=Dm,
                    start=True, stop=True)
            mm.then_inc(sPE, 1)
            # OT[m] = (attn @ V).T halves (fp16), into psA
            E.wait(sA, A_ATTC)
            E.wait(sV, V_VC_D[1])
            for m in range(2):
                if m == 1:
                    E.wait(sV, V_OTC[0])
                for jc in range(4):
                    mm = nc.tensor.matmul(
                        psA[:, m * P : (m + 1) * P],
                        lhsT=V[:, jc, m * P : (m + 1) * P],
                        rhs=attnT[:, jc * P : (jc + 1) * P],
                        start=(jc == 0), stop=(jc == 3))
                mm.then_inc(sPE, 1)
            # out2T halves = WpT16 @ OT (fp16), into psB
            E.wait(sV, V_OTC[1])
            E.wait(sA, A_ATTC)
            for m in range(2):
                if m == 1:
                    E.wait(sV, V_O2B[0])
                for kc in range(2):
                    mm = nc.tensor.matmul(
                        psB[:, m * P : (m + 1) * P],
                        lhsT=WT16p[:, kc, m * P : (m + 1) * P],
                        rhs=OT[:, kc, :], start=(kc == 0), stop=(kc == 1))
                mm.then_inc(sPE, 1)
            # final transpose [dp, i] -> [i, dp] into psA[:, 256:512]
            E.wait(sV, V_O2B[1])
            for m in range(2):
                mm = nc.tensor.transpose(
                    psA[:, C + m * P : C + (m + 1) * P], o2b[:, m, :], ident)
            mm.then_inc(sPE, 1)

        @block.vector
        def _(vector):
            E = EngState(vector)
            for thr, dst, srcap in (
                (PE_XT[1], xT[:, 1, :], psB),
                (PE_WTK[1], WT["k"][:, 1, :], psB[:, 0:C]),
                (PE_KT[1], kT[:, 1, :], psK[:, 1, :]),
                (PE_WTQ[1], WT["q"][:, 1, :], psB[:, 0:C]),
                (PE_QT[1], qT[:, 1, :], psB[:, 0:P]),
                (PE_WTV[1], WT["v"][:, 1, :], psB[:, 0:C]),
                (PE_WTP[1], WT16p[:, 1, :], psB[:, 0:C]),
            ):
                E.wait(sPE, thr)
                nc.vector.tensor_copy(dst, srcap).then_inc(sV, 1)
            # R ops (scalar operands prefetched -> cross + self sync)
            E.wait(sA, 6)
            E.wait(sV, 5)
            for i in range(NQH):        # R (DVE share)
                if not _use_dve(i):
                    continue
                sh, g = divmod(i, 4)
                s, h = divmod(sh, 2)
                q = 32 * g + s
                rank = _DVE_RANK[i]
                if rank > NR_V:
                    E.wait(sPE, pe_main_thr(DVE_ISSUES[rank - 1 - NR_V]))
                ins = nc.vector.tensor_scalar(
                    out=Rv[:, (rank - 1) % NR_V, :], in0=kT[:, h, :],
                    scalar1=qT[:, h, q : q + 1], scalar2=0.0,
                    op0=ALU.add, op1=ALU.max,
                )
                if rank % 2 == 0 or rank == N_DVE_R:
                    ins.then_inc(sV, 1)
            for jc in range(2):         # V0/V1 copies (fp16)
                E.wait(sPE, PE_V[jc])
                nc.vector.tensor_copy(
                    V[:, jc, :], (psA if jc == 0 else psB)[:, 0:C]
                ).then_inc(sV, 1)
            E.wait(sPE, PE_MAIN_DONE)
            nc.vector.tensor_reduce(
                out=negmx, in_=psS, axis=mybir.AxisListType.X,
                op=ALU.max, negate=True,
            ).then_inc(sV, 1)
            E.wait(sA, A_EXP)
            nc.vector.reciprocal(rrec, rsum).then_inc(sV, 1)
            E.wait(sV, V_RREC)          # rrec is a prefetched scalar below
            nc.vector.tensor_scalar(
                out=Dm, in0=ident, scalar1=rrec, scalar2=None, op0=ALU.mult,
            ).then_inc(sV, 1)
            for m in range(2):          # OT copies (fp16)
                E.wait(sPE, PE_OT[m])
                nc.vector.tensor_copy(OT[:, m, :],
                                      psA[:, m * P : (m + 1) * P]
                                      ).then_inc(sV, 1)
            E.wait(sDb, 16)
            for m in range(2):          # out2T + bias -> sbuf
                E.wait(sPE, PE_O2[m])
                nc.vector.tensor_scalar(
                    out=o2b[:, m, :], in0=psB[:, m * P : (m + 1) * P],
                    scalar1=bpt[:, m : m + 1], scalar2=None, op0=ALU.add,
                ).then_inc(sV, 1)

        @block.scalar
        def _(scalar):
            E = EngState(scalar)
            # secondary HWDGE queue: wq, wv, wp, bias
            nc.scalar.dma_start(out=wts["q"],
                                in_=wq.rearrange("(t p) c -> p t c", p=P)
                                ).then_inc(sDq, 16)
            for name, w in (("v", wv), ("p", wp)):
                nc.scalar.dma_start(out=wts[name],
                                    in_=w.rearrange("(t p) c -> p t c", p=P)
                                    ).then_inc(sDvp, 16)
            with nc.allow_non_contiguous_dma(reason="1KB bias load"):
                nc.scalar.dma_start(out=bpt,
                                    in_=bp.rearrange("(h p) -> p h", p=P)
                                    ).then_inc(sDb, 16)
            # preload the exp table set (relu rides along)
            E.wait(sDi, 16)
            nc.scalar.activation(out=scr, in_=ident[:, 0:1], func=AXT.Exp
                                 ).then_inc(sA, 1)
            for thr, dst, srcap in (
                (PE_XT[0], xT[:, 0, :], psA),
                (PE_WTK[0], WT["k"][:, 0, :], psA[:, 0:C]),
                (PE_KT[0], kT[:, 0, :], psK[:, 0, :]),
                (PE_WTQ[0], WT["q"][:, 0, :], psA[:, 0:C]),
                (PE_QT[0], qT[:, 0, :], psA[:, 0:P]),
                (PE_WTV[0], WT["v"][:, 0, :], psA[:, 0:C]),
                (PE_WTP[0], WT16p[:, 0, :], psA[:, 0:C]),
            ):
                E.wait(sPE, thr)
                nc.scalar.copy(dst, srcap).then_inc(sA, 1)
            # R ops: in_ = fp32 kT straight from PSUM (faster + exact add)
            E.wait(sV, 5)
            E.wait(sA, A_PREC)
            E.wait(sPE, PE_KT[1])
            for i in range(NQH):        # R (ACT share)
                if _use_dve(i):
                    continue
                sh, g = divmod(i, 4)
                s, h = divmod(sh, 2)
                q = 32 * g + s
                rank = _ACT_RANK[i]
                if rank > NR_A:
                    E.wait(sPE, pe_main_thr(ACT_ISSUES[rank - 1 - NR_A]))
                ins = nc.scalar.activation(
                    out=Ra[:, (rank - 1) % NR_A, :], in_=psK[:, h, :],
                    func=AXT.Relu, bias=qT[:, h, q : q + 1], scale=1.0,
                )
                if rank % 2 == 0 or rank == N_ACT_R:
                    ins.then_inc(sA, 1)
            for jc in range(2, 4):      # V2/V3 copies (fp16)
                E.wait(sPE, PE_V[jc])
                nc.scalar.copy(V[:, jc, :], psV[:, jc - 2, 0:C]).then_inc(sA, 1)
            E.wait(sPE, PE_MAIN_DONE)
            E.wait(sV, V_NEGMX)
            nc.scalar.activation(
                out=U, in_=psS, func=AXT.Exp, bias=negmx, scale=1.0,
                accum_out=rsum,
            ).then_inc(sA, 1)
            E.wait(sPE, PE_ATT)
            nc.scalar.copy(attnT, psB).then_inc(sA, 1)
            E.wait(sPE, PE_FIN)
            nc.scalar.copy(fin, psA[:, C : 2 * C]).then_inc(sA, 1)



_PROGRAM = None


def build_program():
    global _PROGRAM
    if _PROGRAM is not None:
        return _PROGRAM
    nc = bass.Bass(
        "TRN2", target_bir_lowering=False, debug=False, num_devices=NCORES
    )
    xb = nc.dram_tensor("xb", [N, C], F32, kind="ExternalInput")
    wq = nc.dram_tensor("wq", [C, C], F32, kind="ExternalInput")
    wk = nc.dram_tensor("wk", [C, C], F32, kind="ExternalInput")
    wv = nc.dram_tensor("wv", [C, C], F32, kind="ExternalInput")
    wp = nc.dram_tensor("wp", [C, C], F32, kind="ExternalInput")
    bp = nc.dram_tensor("bp", [C], F32, kind="ExternalInput")
    ident = nc.dram_tensor("ident", [P, P], F32, kind="ExternalInput")
    onesw = nc.dram_tensor("onesw", [P, 64], F16, kind="ExternalInput")
    out = nc.dram_tensor("out", [P, C], F32, kind="ExternalOutput")
    _build_body(nc, xb.ap(), wq.ap(), wk.ap(), wv.ap(), wp.ap(), bp.ap(),
                ident.ap(), onesw.ap(), out.ap())
    _PROGRAM = nc
    return nc


def make_in_maps(x, Wq, Wk, Wv, Wp, bp):
    """Per-core inputs: core = (batch, query-block); x rotated so the core's
    query block is rows 0:128."""
    x = np.ascontiguousarray(np.asarray(x, dtype=np.float32))
    onesw = np.zeros((P, 64), dtype=np.float16)
    onesw[:, 32] = 1.0
    common = {
        "ident": np.eye(P, dtype=np.float32),
        "onesw": onesw,
        "wq": np.ascontiguousarray(np.asarray(Wq, dtype=np.float32)),
        "wk": np.ascontiguousarray(np.asarray(Wk, dtype=np.float32)),
        "wv": np.ascontiguousarray(np.asarray(Wv, dtype=np.float32)),
        "wp": np.ascontiguousarray(np.asarray(Wp, dtype=np.float32)),
        "bp": np.ascontiguousarray(np.asarray(bp, dtype=np.float32)),
    }
    in_maps = []
    for core in range(NCORES):
        b, qb = divmod(core, NCORES // B)
        xrot = np.ascontiguousarray(np.roll(x[b], -qb * P, axis=0))
        in_maps.append({"xb": xrot, **common})
    return in_maps


def assemble(results):
    out = np.zeros((B, N, C), dtype=np.float32)
    for core in range(NCORES):
        b, qb = divmod(core, NCORES // B)
        out[b, qb * P : (qb + 1) * P] = results[core]["out"]
    return out


def kernel(x, Wq, Wk, Wv, Wp, bp):
    nc = build_program()
    in_maps = make_in_maps(x, Wq, Wk, Wv, Wp, bp)
    res = run_bass_kernel_spmd(nc, in_maps, list(range(NCORES)))
    return assemble(res.results)


if __name__ == "__main__":
    rng = np.random.default_rng(0)
    inputs = {
        "x": rng.standard_normal((B, N, C), dtype=np.float32),
        "Wq": rng.standard_normal((C, C), dtype=np.float32) * 0.02,
        "Wk": rng.standard_normal((C, C), dtype=np.float32) * 0.02,
        "Wv": rng.standard_normal((C, C), dtype=np.float32) * 0.02,
        "Wp": rng.standard_normal((C, C), dtype=np.float32) * 0.02,
        "bp": rng.standard_normal((C,), dtype=np.float32) * 0.02,
    }
    out = kernel(**inputs)
    print(out.shape, out.dtype)
